# revision 1
# baseline (speedup 1.0000x reference)
"""Distance-correlation (DcorLoss) kernel for 8 trn2 NeuronCores.

Math: for x, y [n=8192, d=128]:
  a = pairwise_dist(x), b = pairwise_dist(y)   (n x n, symmetric, zero diag)
  A = double_center(a), B = double_center(b)
  dcor = -sqrt(sum(A*B)) / sqrt(sqrt(sum(A*A)) * sqrt(sum(B*B)))   (n^2 factors cancel)

Key identities (never materialize A/B):
  sum(HaH o HbH) = sum(at o bt) - 2/n * dot(rs_at, rs_bt) + sum(at)*sum(bt)/n^2
for at = a - mu (any constant shift; double centering annihilates it). The
mu ~ E[dist] shift keeps fp32 device accumulations well-conditioned. And the
squared-distance Frobenius norms have a closed form -- no elementwise pass:
  sum_ij dist^2_ij = 2n * sum_i |x_i|^2 - 2 |sum_i x_i|^2
so only sum (a-mu)*b needs streaming; sum(a-mu)^2 / sum(b-mu)^2 come from
row sums + norms + the column-sum vector of x. Cross-core combining is fp64
on host (the only inter-core step; partials are tiny).

Sharding: block-rows; core c owns rows [c*1024, (c+1)*1024), streams all columns.

Per (128-row x 1024-col) tile pair, the device computes:
  PE:   psum = -2*x_blk^T x (K=128, bf16) + ones2 (x) [n_hi; n_lo] (K=2 bf16
        hi/lo split of the fp32 column norms -> ~16-bit mantissa)
  DVE:  psum[diag block] += mu^2 * I  (data-driven: per-core `diagsel` input
        is nonzero only on the core's own diagonal window)
  ACT:  t = sqrt(psum + n_i)  [per-partition fp32 bias], accum_out -> row sums
  DVE:  (t_a - mu) * t_b -> accum_out   (one scalar_tensor_tensor)
Forcing the diagonal of sq to mu^2 keeps sqrt NaN-free; host replaces the known
diagonal contribution exactly (true diag of a is 0).
"""

import os

import numpy as np

import concourse.bass as bass
import concourse.tile as tile
from concourse import bacc, mybir
from concourse.bass_utils import run_bass_kernel_spmd

P = 128            # partitions / d
N = 8192           # points
NCORES = 8
BLK = N // NCORES  # 1024 rows per core
CI_N = BLK // P    # 8 row chunks per core
W = 1024           # column window
JT_N = N // W      # 8 column windows
MU = 16.0          # ~E[pairwise dist] for randn d=128; any constant is exact math
MU2 = MU * MU
RES_W = 48

_programs = {}


def _build(mm_mode: str):
    """mm_mode: 'bf16' | 'f32' (matmul operand dtype)."""
    dt = mybir.dt
    f32 = dt.float32
    mmdt = dt.bfloat16 if mm_mode == "bf16" else dt.float32
    A = mybir.AluOpType
    AF = mybir.ActivationFunctionType

    nc = bacc.Bacc("TRN2", target_bir_lowering=False, debug=False,
                   num_devices=NCORES)

    dxT = nc.dram_tensor("xT", [P, N], f32, kind="ExternalInput").ap()
    dyT = nc.dram_tensor("yT", [P, N], f32, kind="ExternalInput").ap()
    dxb = nc.dram_tensor("xblkT", [P, BLK], f32, kind="ExternalInput").ap()
    dyb = nc.dram_tensor("yblkT", [P, BLK], f32, kind="ExternalInput").ap()
    ddg = nc.dram_tensor("diagsel", [P, JT_N * P], f32, kind="ExternalInput").ap()
    dew = nc.dram_tensor("eyewide", [P, 4 * 512], f32, kind="ExternalInput").ap()
    dout = nc.dram_tensor("out", [P, RES_W], f32, kind="ExternalOutput").ap()

    with tile.TileContext(nc) as tc:
        with tc.tile_pool(name="const", bufs=1) as cp, \
             tc.tile_pool(name="psum", bufs=1, space="PSUM") as pp, \
             tc.tile_pool(name="ab", bufs=3) as abp, \
             tc.tile_pool(name="trd", bufs=2) as trd:

            # ── persistent operands ────────────────────────────────────
            xTc = cp.tile([P, N], mmdt, tag="xTc")
            yTc = cp.tile([P, N], mmdt, tag="yTc")
            xblk2 = cp.tile([P, BLK], mmdt, tag="xblk2")
            yblk2 = cp.tile([P, BLK], mmdt, tag="yblk2")
            # column norms as bf16 hi/lo rows: n_j = hi_j + lo_j (K=2 matmul)
            nfhl_x = cp.tile([2, N], mmdt, tag="nfhl_x")
            nfhl_y = cp.tile([2, N], mmdt, tag="nfhl_y")
            nbx = cp.tile([P, CI_N], f32, tag="nbx")
            nby = cp.tile([P, CI_N], f32, tag="nby")
            diag_m = cp.tile([P, JT_N * P], mmdt, tag="diag_m")
            eyew_m = cp.tile([P, 4 * 512], mmdt, tag="eyew_m")
            ones2 = cp.tile([2, P], mmdt, tag="ones2")
            nc.vector.memset(ones2[:], 1.0)
            onesc = cp.tile([P, 1], mmdt, tag="onesc")
            nc.vector.memset(onesc[:], 1.0)
            quarc = cp.tile([P, 1], f32, tag="quarc")
            nc.vector.memset(quarc[:], 0.25)

            res = cp.tile([P, RES_W], f32, tag="res")
            nc.vector.memset(res[:], 0.0)

            # PE warm-up: ~5us of dense back-to-back matmuls on constant data
            # so the HAM clock-gate reaches 8/8 before real work starts (cold
            # first executions otherwise run the whole kernel at half clock)
            wur = cp.tile([2, 512], mmdt, tag="wur")
            nc.vector.memset(wur[:], 0.0)
            wt = pp.tile([P, W], f32, tag="a", bufs=2)
            for _ in range(24):
                nc.tensor.matmul(wt[:, 0:512], ones2[:], wur[:],
                                 start=True, stop=True)

            # ── setup (scratch pool closes before the main loop) ──────
            with tc.tile_pool(name="setup", bufs=1) as sp:
                big = sp.tile([P, N], f32, tag="big")
                nc.sync.dma_start(big[:], dxT[:])
                nc.scalar.copy(xTc[:], big[:])
                big2 = sp.tile([P, N], f32, tag="big")
                nc.sync.dma_start(big2[:], dyT[:])
                nc.scalar.copy(yTc[:], big2[:])

                for dsrc, dst in ((dxb, xblk2), (dyb, yblk2)):
                    raw = sp.tile([P, BLK], f32, tag="braw")
                    nc.sync.dma_start(raw[:], dsrc[:])
                    nc.scalar.mul(dst[:], raw[:], -2.0)

                dgr = sp.tile([P, JT_N * P], f32, tag="dgr")
                nc.sync.dma_start(dgr[:], ddg[:])
                nc.scalar.copy(diag_m[:], dgr[:])
                ewr = sp.tile([P, 4 * 512], f32, tag="ewr")
                nc.sync.dma_start(ewr[:], dew[:])
                nc.scalar.copy(eyew_m[:], ewr[:])

                # full column norms nf[j] = sum_d xTc[d, j]^2 (fp32 in PSUM),
                # split per-slice into bf16 hi/lo; DMA does the partition
                # placement into nfhl (engines are lane-aligned)
                for src, nfhl in ((xTc, nfhl_x), (yTc, nfhl_y)):
                    hi = sp.tile([1, N], mmdt, tag="hi")
                    lo = sp.tile([1, N], mmdt, tag="lo")
                    for j8 in range(JT_N):
                        sq = sp.tile([P, W], mmdt, tag="sq", bufs=2)
                        nc.scalar.activation(sq[:], src[:, bass.ts(j8, W)],
                                             AF.Square)
                        for h in range(2):
                            ps = pp.tile([P, W], f32, tag="a", bufs=2)
                            sl = bass.ds(j8 * W + h * 512, 512)
                            nc.tensor.matmul(ps[0:1, 0:512], onesc[:],
                                             sq[:, bass.ts(h, 512)],
                                             start=True, stop=True)
                            nc.vector.tensor_copy(hi[0:1, sl],
                                                  ps[0:1, 0:512])
                            nc.vector.tensor_tensor(lo[0:1, sl],
                                                    ps[0:1, 0:512],
                                                    hi[0:1, sl],
                                                    op=A.subtract)
                        # ship this window now so main-loop iterations can
                        # start before the whole norms row is built
                        wsl = bass.ts(j8, W)
                        nc.sync.dma_start(nfhl[0:1, wsl], hi[0:1, wsl])
                        nc.sync.dma_start(nfhl[1:2, wsl], lo[0:1, wsl])

                # block norms as [P, CI_N] fp32 columns (sqrt bias):
                # (-2x)^2 * 0.25 = x^2
                for src, dst in ((xblk2, nbx), (yblk2, nby)):
                    sqb = sp.tile([P, BLK], f32, tag="sqb")
                    nc.scalar.activation(sqb[:], src[:], AF.Square)
                    for ci in range(CI_N):
                        ps = pp.tile([P, W], f32, tag="a", bufs=2)
                        nc.tensor.matmul(ps[:, 0:1], sqb[:, bass.ts(ci, P)],
                                         quarc[:], start=True, stop=True)
                        nc.vector.tensor_copy(dst[:, ci:ci + 1], ps[:, 0:1])

            # closed-form helpers: sum of norms (hi+lo rows reduced) and the
            # column-sum vector s = sum_i x_i, both over the bf16 values
            nc.vector.tensor_reduce(res[0:2, 41:42], nfhl_x[:, :],
                                    axis=mybir.AxisListType.X, op=A.add)
            nc.vector.tensor_reduce(res[0:2, 42:43], nfhl_y[:, :],
                                    axis=mybir.AxisListType.X, op=A.add)
            nc.vector.tensor_reduce(res[:, 44:45], xTc[:, :],
                                    axis=mybir.AxisListType.X, op=A.add)
            nc.vector.tensor_reduce(res[:, 45:46], yTc[:, :],
                                    axis=mybir.AxisListType.X, op=A.add)

            # ── stages ────────────────────────────────────────────────
            st = [cp.tile([P, CI_N * JT_N], f32, tag=f"st{q}", name=f"st{q}")
                  for q in range(3)]

            # ── main loop ─────────────────────────────────────────────
            for ci in range(CI_N):
                for jt in range(JT_N):
                    col = ci * JT_N + jt
                    h_diag = ci // 4
                    psA = pp.tile([P, W], f32, tag="a", bufs=2)
                    psB = pp.tile([P, W], f32, tag="b", bufs=2)
                    # weight-grouped order: mains (xblk2 / yblk2), then all
                    # norm matmuls (shared ones2 weights), then the diag eye
                    for ps_, blk2, full in ((psA, xblk2, xTc),
                                            (psB, yblk2, yTc)):
                        for h in range(2):
                            nc.tensor.matmul(
                                ps_[:, bass.ds(h * 512, 512)],
                                blk2[:, bass.ts(ci, P)],
                                full[:, bass.ds(jt * W + h * 512, 512)],
                                start=True, stop=False)
                    for ps_, nfhl in ((psA, nfhl_x), (psB, nfhl_y)):
                        for h in range(2):
                            nc.tensor.matmul(
                                ps_[:, bass.ds(h * 512, 512)], ones2[:],
                                nfhl[:, bass.ds(jt * W + h * 512, 512)],
                                start=False, stop=(h != h_diag))
                    for ps_ in (psA, psB):
                        # += mu^2*I on the diag window (zeros unless jt ==
                        # core id): (mu I)^T (mu I @ offset)
                        nc.tensor.matmul(ps_[:, bass.ds(h_diag * 512, 512)],
                                         diag_m[:, bass.ts(jt, P)],
                                         eyew_m[:, bass.ts(ci % 4, 512)],
                                         start=False, stop=True)

                    aT = abp.tile([P, W], f32, tag="a")
                    bT = abp.tile([P, W], f32, tag="b")
                    nc.scalar.activation(aT[:], psA[:], AF.Sqrt,
                                         bias=nbx[:, ci:ci + 1],
                                         accum_out=st[0][:, col:col + 1])
                    nc.scalar.activation(bT[:], psB[:], AF.Sqrt,
                                         bias=nby[:, ci:ci + 1],
                                         accum_out=st[1][:, col:col + 1])
                    t0 = trd.tile([P, W], f32, tag="t")
                    nc.vector.scalar_tensor_tensor(
                        t0[:], aT[:], MU, bT[:], op0=A.subtract, op1=A.mult,
                        accum_out=st[2][:, col:col + 1])

            # ── epilogue ──────────────────────────────────────────────
            nc.vector.tensor_copy(res[:, 24:24 + CI_N], nbx[:, :])
            nc.vector.tensor_copy(res[:, 32:32 + CI_N], nby[:, :])
            for q in range(3):
                for ci in range(CI_N):
                    o = q * CI_N + ci
                    nc.vector.tensor_reduce(res[:, o:o + 1],
                                            st[q][:, bass.ts(ci, JT_N)],
                                            axis=mybir.AxisListType.X,
                                            op=A.add)
            nc.sync.dma_start(dout[:], res[:])

    nc.compile()
    return nc


def _get_program(mm_mode: str):
    if mm_mode not in _programs:
        _programs[mm_mode] = _build(mm_mode)
    return _programs[mm_mode]


def make_in_maps(x: np.ndarray, y: np.ndarray):
    x = np.ascontiguousarray(np.asarray(x, np.float32))
    y = np.ascontiguousarray(np.asarray(y, np.float32))
    xT = np.ascontiguousarray(x.T)
    yT = np.ascontiguousarray(y.T)
    eye = (np.eye(P, dtype=np.float32) * MU)
    ew = np.zeros((P, 4 * 512), np.float32)
    for k in range(4):
        for p in range(P):
            ew[p, k * 512 + k * P + p] = MU
    in_maps = []
    for c in range(NCORES):
        dg = np.zeros((P, JT_N * P), np.float32)
        dg[:, c * P:(c + 1) * P] = eye
        in_maps.append({
            "xT": xT,
            "yT": yT,
            "xblkT": np.ascontiguousarray(x[c * BLK:(c + 1) * BLK].T),
            "yblkT": np.ascontiguousarray(y[c * BLK:(c + 1) * BLK].T),
            "diagsel": dg,
            "eyewide": ew,
        })
    return in_maps


def finalize(outs):
    """outs: list of 8 [128, 48] arrays -> scalar dcor (fp64 host math).

    Cols: rsa 0:8 | rsb 8:16 | pab 16:24 | [0:2,41]=(sum hi, sum lo) of x
    norms | [0:2,42]= same for y | [:,44]=sum_i x_i | [:,45]=sum_i y_i.
    Device row sums include the forced diag ~mu (true diag of a is 0).
    """
    n = float(N)
    rs_a = np.empty(N, np.float64)
    rs_b = np.empty(N, np.float64)
    pab = 0.0
    for c, o in enumerate(outs):
        o = np.asarray(o, np.float64)
        rs_a[c * BLK:(c + 1) * BLK] = o[:, 0:CI_N].T.ravel()
        rs_b[c * BLK:(c + 1) * BLK] = o[:, CI_N:2 * CI_N].T.ravel()
        pab += o[:, 2 * CI_N:3 * CI_N].sum()

    o0 = np.asarray(outs[0], np.float64)
    # column-norm sums as the device's K=2 matmul sees them (bf16 hi+lo of
    # bf16-rounded squares); row-bias norms are the fp32-exact path
    sum_nxc = o0[0, 41] + o0[1, 41]
    sum_nyc = o0[0, 42] + o0[1, 42]
    sum_nxr = sum(np.asarray(o, np.float64)[:, 24:24 + CI_N].sum()
                  for o in outs)
    sum_nyr = sum(np.asarray(o, np.float64)[:, 32:32 + CI_N].sum()
                  for o in outs)
    sx = o0[:, 44]                        # sum_i x_i  [128]
    sy = o0[:, 45]
    # closed-form squared-distance Frobenius norms, consistent with the
    # device's mixed n_i/n_j paths (true zero diag):
    sq_a = n * (sum_nxr + sum_nxc) - 2.0 * np.dot(sx, sx)   # sum_ij a_ij^2
    sq_b = n * (sum_nyr + sum_nyc) - 2.0 * np.dot(sy, sy)

    sa = rs_a - MU          # true (zero-diag) row sums of a
    sb = rs_b - MU
    sat = sa - n * MU       # row sums of (a - mu)
    sbt = sb - n * MU
    Ua = sat.sum()
    Ub = sbt.sum()
    # device pab = sum (a-mu)*b (diag contributes ~0 in device and truth)
    Sab = pab - MU * (sa.sum() - MU * n * n)
    Saa = sq_a - 2.0 * MU * sa.sum() + MU2 * n * n
    Sbb = sq_b - 2.0 * MU * sb.sum() + MU2 * n * n

    sumAB = Sab - 2.0 * np.dot(sat, sbt) / n + Ua * Ub / n**2
    sumAA = Saa - 2.0 * np.dot(sat, sat) / n + Ua * Ua / n**2
    sumBB = Sbb - 2.0 * np.dot(sbt, sbt) / n + Ub * Ub / n**2

    inv_n2 = 1.0 / (n * n)
    dcov2_xy = sumAB * inv_n2
    dcov2_xx = sumAA * inv_n2
    dcov2_yy = sumBB * inv_n2
    dcor = -np.sqrt(dcov2_xy) / np.sqrt(np.sqrt(dcov2_xx) * np.sqrt(dcov2_yy))
    return np.asarray(dcor, dtype=np.float32)


def run(x, y, mm_mode=None, trace=False, tmpdir=None):
    if mm_mode is None:
        mm_mode = os.environ.get("DCOR_MM", "bf16")
    nc = _get_program(mm_mode)
    in_maps = make_in_maps(x, y)
    res = run_bass_kernel_spmd(nc, in_maps, core_ids=list(range(NCORES)),
                               trace=trace, tmpdir=tmpdir)
    outs = [r["out"] for r in res.results]
    return finalize(outs), res


def kernel(x, y):
    val, _ = run(x, y)
    return val



# revision 20
# speedup vs baseline: 1.4391x; 1.4391x over previous
"""Distance-correlation (DcorLoss) kernel for 8 trn2 NeuronCores.

Math: for x, y [n=8192, d=128]:
  a = pairwise_dist(x), b = pairwise_dist(y)   (n x n, symmetric, zero diag)
  A = double_center(a), B = double_center(b)
  dcor = -sqrt(sum(A*B)) / sqrt(sqrt(sum(A*A)) * sqrt(sum(B*B)))

Identities (never materialize A/B):
  sum(HaH o HbH) = sum(at o bt) - 2/n * dot(rs_at, rs_bt) + sum(at)*sum(bt)/n^2
with at = a - mu. sum(a-mu)^2 via the closed form for sum a^2 = sum sq.
Only sum (a-mu)*b and the row sums of a/b need streaming the matrices.

Symmetric block coverage: core c owns row block c (1024 rows). Each unordered
block pair {r, j} is computed once: core c runs 5 column-window "slots"
s=0..4 over windows (c+s) mod 8. Slot 0 = diagonal block, slots 1-3 pairs
counted twice on host, slot 4 pair computed by both ends (counted once each).
Row sums for the mirrored (uncomputed) windows of block c come from COLUMN
sums of slots 1-3 tiles of cores (c+5..c+7) mod 8, computed on-device with
ones^T matmuls accumulated in PSUM across the 8 row chunks of a slot.

Per (128-row x 1024-col) tile pair, the device computes:
  PE:   psum = -2*x_blk^T x  (fp32r, full speed at 512 moving cols)
        + column norms via fp8e4 DoubleRow matmuls (4-term hi/lo split rows)
        + mu^2*I on the slot-0 diagonal sub-block (fp8 DoubleRow)
  ACT:  t = sqrt(psum + n_i)  [per-partition fp32 row-norm bias]
  DVE:  row-sum reduces of t_a, t_b -> res columns
  POOL: (t_a - mu) * t_b with accum -> res columns (gpsimd engine)
  PE:   ones^T t_a / ones^T t_b column sums (slots 1-3, fp32r) -> PSUM
Cross-core combining is fp64 on host (partials are tiny).
"""

import os

import numpy as np
import ml_dtypes

import concourse.bass as bass
import concourse.tile as tile
from concourse import bacc, mybir
from concourse.bass_utils import run_bass_kernel_spmd

P = 128            # partitions / d
N = 8192           # points
NCORES = 8
BLK = N // NCORES  # 1024 rows per core
CI_N = BLK // P    # 8 row chunks per core
W = 1024           # column window
NSLOT = 5          # symmetric coverage slots
MU = 16.0
F8 = ml_dtypes.float8_e4m3
BF = ml_dtypes.bfloat16

_programs = {}


def _build():
    dt = mybir.dt
    f32 = dt.float32
    f32r = dt.float32r
    f8 = dt.float8e4
    bf = dt.bfloat16
    A = mybir.AluOpType
    AF = mybir.ActivationFunctionType
    DR = mybir.MatmulPerfMode.DoubleRow

    nc = bacc.Bacc("TRN2", target_bir_lowering=False, debug=False,
                   num_devices=NCORES)

    dxT = nc.dram_tensor("xT", [P, NSLOT * W], bf, kind="ExternalInput").ap()
    dyT = nc.dram_tensor("yT", [P, NSLOT * W], bf, kind="ExternalInput").ap()
    dxb = nc.dram_tensor("xb2", [P, BLK], bf, kind="ExternalInput").ap()
    dyb = nc.dram_tensor("yb2", [P, BLK], bf, kind="ExternalInput").ap()
    drn = nc.dram_tensor("rn", [P, 2 * CI_N], f32, kind="ExternalInput").ap()
    dcnx = nc.dram_tensor("cnx", [2, 2 * NSLOT * W], f8, kind="ExternalInput").ap()
    dcny = nc.dram_tensor("cny", [2, 2 * NSLOT * W], f8, kind="ExternalInput").ap()
    dey = nc.dram_tensor("eyew", [P, 2 * 384], f8, kind="ExternalInput").ap()
    dres = nc.dram_tensor("res", [P, 128], f32, kind="ExternalOutput").ap()
    dcols = nc.dram_tensor("cols", [1, 12 * 512], f32, kind="ExternalOutput").ap()

    with tile.TileContext(nc) as tc:
        with tc.tile_pool(name="const", bufs=1) as cp, \
             tc.tile_pool(name="psum", bufs=1, space="PSUM") as pp, \
             tc.tile_pool(name="ab", bufs=3) as abp, \
             tc.tile_pool(name="trd", bufs=2) as trd:

            # ── persistent operands ────────────────────────────────────
            xTt = cp.tile([P, NSLOT * W], bf, tag="xTt")
            yTt = cp.tile([P, NSLOT * W], bf, tag="yTt")
            xb2 = cp.tile([P, BLK], bf, tag="xb2")
            yb2 = cp.tile([P, BLK], bf, tag="yb2")
            rnt = cp.tile([P, 2 * CI_N], f32, tag="rnt")
            cntx = cp.tile([2, 2, NSLOT * W], f8, tag="cntx")
            cnty = cp.tile([2, 2, NSLOT * W], f8, tag="cnty")
            eyew = cp.tile([P, 2, 384], f8, tag="eyew")
            cnw = cp.tile([2, 2, P], f8, tag="cnw")       # all-ones DR lhsT
            nc.vector.memset(cnw[:], 1.0)
            onesf = cp.tile([P, 1], f32, tag="onesf")
            nc.vector.memset(onesf[:], 1.0)
            ones1 = cp.tile([P, 1], f32r, tag="ones1")   # colsum lhsT (f32r)
            nc.vector.tensor_copy(ones1[:], onesf[:])
            res = cp.tile([P, 128], f32, tag="res")
            nc.vector.memset(res[:], 0.0)
            colstage = cp.tile([1, 12 * 512], f32, tag="colstage")
            nc.vector.memset(colstage[:], 0.0)

            # ACT sqrt-table preload (avoid a mid-loop ACT_TABLE_LOAD)
            sone = cp.tile([P, 1], f32, tag="sone")
            nc.vector.memset(sone[:], 1.0)
            sdum = cp.tile([P, 1], f32, tag="sdum")
            nc.scalar.activation(sdum[:], sone[:], AF.Sqrt)

            # PE warm-up: dense back-to-back matmuls on constant data so the
            # HAM clock reaches full speed before real work starts
            wur = cp.tile([2, 512], bf, tag="wur")
            nc.vector.memset(wur[:], 0.0)
            ones2b = cp.tile([2, P], bf, tag="ones2b")
            nc.vector.memset(ones2b[:], 1.0)
            wt = pp.tile([P, W], f32, tag="a", bufs=1)
            for _ in range(24):
                nc.tensor.matmul(wt[:, 0:512], ones2b[:], wur[:],
                                 start=True, stop=True)

            # ── input DMAs (small/critical first; windows stream in) ──
            nc.sync.dma_start(rnt[:], drn[:])
            nc.sync.dma_start(cntx[:], dcnx[:])
            nc.sync.dma_start(cnty[:], dcny[:])
            nc.sync.dma_start(eyew[:], dey[:])
            nc.sync.dma_start(xb2[:], dxb[:])
            nc.sync.dma_start(yb2[:], dyb[:])
            for s in range(NSLOT):
                sl = bass.ts(s, W)
                nc.sync.dma_start(xTt[:, sl], dxT[:, sl])
                nc.sync.dma_start(yTt[:, sl], dyT[:, sl])

            # ── main loop ─────────────────────────────────────────────
            for s in range(NSLOT):
                cst = None
                if 1 <= s <= 3:
                    cst = [pp.tile([1, 512], f32, tag=f"cs{k}", bufs=1,
                                   name=f"cs{k}")
                           for k in range(4)]
                for ci in range(CI_N):
                    col = s * CI_N + ci
                    psA = pp.tile([P, W], f32, tag="a", bufs=1)
                    psB = pp.tile([P, W], f32, tag="b", bufs=1)
                    for ps_, blk2, full, cnt in ((psA, xb2, xTt, cntx),
                                                 (psB, yb2, yTt, cnty)):
                        for h in range(2):
                            nc.tensor.matmul(
                                ps_[:, bass.ds(h * 512, 512)],
                                blk2[:, bass.ts(ci, P)],
                                full[:, bass.ds(s * W + h * 512, 512)],
                                start=True, stop=False)
                        if s == 0:
                            # += mu^2*I on this chunk's diagonal sub-block
                            qd = ci // 2
                            off = 128 * ((ci + 1) % 2)
                            nc.tensor.matmul(
                                ps_[:, bass.ds(qd * 256, 256)],
                                eyew[:, :, 128:256],
                                eyew[:, :, bass.ds(off, 256)],
                                start=False, stop=False, perf_mode=DR)
                        for q in range(4):
                            nc.tensor.matmul(
                                ps_[:, bass.ds(q * 256, 256)],
                                cnw[:],
                                cnt[:, :, bass.ds(s * W + q * 256, 256)],
                                start=False, stop=True, perf_mode=DR)

                    aT = abp.tile([P, W], f32r, tag="a")
                    bT = abp.tile([P, W], f32r, tag="b")
                    nc.scalar.activation(aT[:], psA[:], AF.Sqrt,
                                         bias=rnt[:, ci:ci + 1],
                                         accum_out=res[:, col:col + 1])
                    nc.scalar.activation(bT[:], psB[:], AF.Sqrt,
                                         bias=rnt[:, CI_N + ci:CI_N + ci + 1])
                    nc.vector.tensor_reduce(res[:, 40 + col:41 + col], bT[:],
                                            axis=mybir.AxisListType.X, op=A.add)
                    t0 = trd.tile([P, W], f32, tag="t")
                    nc.vector.scalar_tensor_tensor(
                        t0[:], aT[:], MU, bT[:], op0=A.subtract, op1=A.mult,
                        accum_out=res[:, 80 + col:81 + col])
                    if cst is not None:
                        for r, (ssrc, h) in enumerate(
                                ((aT, 0), (aT, 1), (bT, 0), (bT, 1))):
                            nc.tensor.matmul(
                                cst[r][:],
                                ones1[:],
                                ssrc[:, bass.ts(h, 512)],
                                start=(ci == 0), stop=(ci == CI_N - 1),
                                skip_group_check=True,
                                tile_position=(0, 0))
                if cst is not None:
                    for r in range(4):
                        off = (s - 1) * 2048 + r * 512
                        nc.vector.tensor_copy(
                            colstage[0:1, bass.ds(off, 512)],
                            cst[r][:])

            nc.sync.dma_start(dres[:], res[:])
            nc.sync.dma_start(dcols[:], colstage[:])

    nc.compile()
    return nc


def _get_program(mm_mode="f32r"):
    if mm_mode not in _programs:
        _programs[mm_mode] = _build()
    return _programs[mm_mode]


def _fp8_terms(v, k=4):
    """Successive fp8e4m3 split: v ~= sum of k fp8-representable terms."""
    r = np.asarray(v, np.float64).copy()
    terms = []
    for _ in range(k):
        t = r.astype(F8).astype(np.float64)
        terms.append(t)
        r -= t
    return terms


def _host_quant(x):
    """Per-matrix host-side quantities (fp64): norms and fp8 colnorm terms.

    The device consumes bf16(x); all norms come from those exact values."""
    x64 = np.asarray(x, np.float32).astype(BF).astype(np.float64)
    n_exact = (x64 * x64).sum(1)                       # [N]
    rn = n_exact.astype(np.float32).astype(np.float64)  # shipped fp32 bias
    terms = _fp8_terms(n_exact, 4)
    cn = terms[0] + terms[1] + terms[2] + terms[3]
    return n_exact, rn, terms, cn


def make_in_maps(x, y):
    x = np.ascontiguousarray(np.asarray(x, np.float32))
    y = np.ascontiguousarray(np.asarray(y, np.float32))
    _, rnx, tx, _ = _host_quant(x)
    _, rny, ty, _ = _host_quant(y)
    xT = x.astype(BF).T  # [128, 8192] bf16
    yT = y.astype(BF).T

    # eyew[p, 0, k] = 16*delta(p == k-128); plane 1 zero
    eyew = np.zeros((P, 2, 384), np.float32)
    for p in range(P):
        eyew[p, 0, p + 128] = MU
    eyew8 = eyew.astype(F8).reshape(P, 2 * 384)

    in_maps = []
    for c in range(NCORES):
        wins = [(c + s) % NCORES for s in range(NSLOT)]
        colsel = np.concatenate([np.arange(w * W, (w + 1) * W) for w in wins])
        rn = np.empty((P, 2 * CI_N), np.float32)
        for ci in range(CI_N):
            base = c * BLK + ci * P
            rn[:, ci] = rnx[base:base + P]
            rn[:, CI_N + ci] = rny[base:base + P]

        def cn_pack(terms):
            # [2, 2, NSLOT*W]: (p,t) = (0,0)c0 (1,0)c1 (0,1)c2 (1,1)c3
            out = np.zeros((2, 2, NSLOT * W), np.float32)
            out[0, 0] = terms[0][colsel]
            out[1, 0] = terms[1][colsel]
            out[0, 1] = terms[2][colsel]
            out[1, 1] = terms[3][colsel]
            return out.astype(F8).reshape(2, 2 * NSLOT * W)

        in_maps.append({
            "xT": np.ascontiguousarray(xT[:, colsel]),
            "yT": np.ascontiguousarray(yT[:, colsel]),
            "xb2": np.ascontiguousarray(
                (-2.0 * xT[:, c * BLK:(c + 1) * BLK].astype(np.float32))
                .astype(BF)),
            "yb2": np.ascontiguousarray(
                (-2.0 * yT[:, c * BLK:(c + 1) * BLK].astype(np.float32))
                .astype(BF)),
            "rn": rn,
            "cnx": cn_pack(tx),
            "cny": cn_pack(ty),
            "eyew": eyew8,
        })
    return in_maps


def finalize(outs, x, y):
    """outs: list of 8 dicts with 'res' [128,128] and 'cols' [4, 3072].

    res cols: rs_a 0:40 | rs_b 40:80 | pab 80:120, col = s*8+ci, value at
    partition p belongs to row c*1024+ci*128+p.
    cols rows: 0/1 = a-tile column sums (halves 0/1), 2/3 = same for b;
    slot s occupies cols (s-1)*512 : s*512.
    """
    n = float(N)
    nx, rnx, _, cnx = _host_quant(x)
    ny, rny, _, cny = _host_quant(y)
    x64 = np.asarray(x, np.float32).astype(BF).astype(np.float64)
    y64 = np.asarray(y, np.float32).astype(BF).astype(np.float64)

    res = [np.asarray(o["res"], np.float64) for o in outs]
    cols = [np.asarray(o["cols"], np.float64) for o in outs]

    rs_a = np.empty(N)
    rs_b = np.empty(N)
    pab = 0.0
    wslot = np.array([1.0, 2.0, 2.0, 2.0, 1.0])
    for c in range(NCORES):
        r = res[c]
        st0 = r[:, 0:40].reshape(P, NSLOT, CI_N)    # [p, s, ci]
        st1 = r[:, 40:80].reshape(P, NSLOT, CI_N)
        st2 = r[:, 80:120].reshape(P, NSLOT, CI_N)
        own_a = st0.sum(axis=1)                     # [p, ci]
        own_b = st1.sum(axis=1)
        # mirrored contributions: window (c+d)%8, d=5,6,7 -> core m slot 8-d
        mir_a = np.zeros(BLK)
        mir_b = np.zeros(BLK)
        for d in (5, 6, 7):
            m = (c + d) % NCORES
            sp = 8 - d
            base = (sp - 1) * 2048
            cv = cols[m][0]
            mir_a += np.concatenate([cv[base:base + 512],
                                     cv[base + 512:base + 1024]])
            mir_b += np.concatenate([cv[base + 1024:base + 1536],
                                     cv[base + 1536:base + 2048]])
        blk_a = own_a.T.ravel() + mir_a             # [1024], ci-major
        blk_b = own_b.T.ravel() + mir_b
        rs_a[c * BLK:(c + 1) * BLK] = blk_a
        rs_b[c * BLK:(c + 1) * BLK] = blk_b
        pab += (st2.sum(axis=(0, 2)) * wslot).sum()

    # closed-form sums of device sq over all ij (fp64, host-exact)
    sum_sq_a = n * rnx.sum() + n * cnx.sum() - 2.0 * (x64.sum(0) @ x64.sum(0))
    sum_sq_b = n * rny.sum() + n * cny.sum() - 2.0 * (y64.sum(0) @ y64.sum(0))
    diag_sq_a = (rnx + cnx - 2.0 * nx).sum()
    diag_sq_b = (rny + cny - 2.0 * ny).sum()

    sa = rs_a - MU          # true rowsums (device diag sqrt(256+eps) ~ 16)
    sb = rs_b - MU
    Sq_a_off = sum_sq_a - diag_sq_a
    Sq_b_off = sum_sq_b - diag_sq_b
    sat = sa - n * MU
    sbt = sb - n * MU
    Ua, Ub = sat.sum(), sbt.sum()
    # device pab = weighted sum of (a-mu)*b; forced diag contributes
    # (16-16)*16 = 0, matching the true (0-mu)*0 = 0.
    # Sab = sum over all ij of (a_true - mu)(b_true - mu)
    Sab = pab - MU * (sa.sum() - MU * n * n)
    Saa = Sq_a_off - 2.0 * MU * sa.sum() + MU * MU * n * n
    Sbb = Sq_b_off - 2.0 * MU * sb.sum() + MU * MU * n * n

    sumAB = Sab - 2.0 * np.dot(sat, sbt) / n + Ua * Ub / n ** 2
    sumAA = Saa - 2.0 * np.dot(sat, sat) / n + Ua * Ua / n ** 2
    sumBB = Sbb - 2.0 * np.dot(sbt, sbt) / n + Ub * Ub / n ** 2

    inv_n2 = 1.0 / (n * n)
    dcor = (-np.sqrt(sumAB * inv_n2)
            / np.sqrt(np.sqrt(sumAA * inv_n2) * np.sqrt(sumBB * inv_n2)))
    return np.asarray(dcor, dtype=np.float32)


def run(x, y, mm_mode=None, trace=False, tmpdir=None):
    nc = _get_program()
    in_maps = make_in_maps(x, y)
    res = run_bass_kernel_spmd(nc, in_maps, core_ids=list(range(NCORES)),
                               trace=trace, tmpdir=tmpdir)
    return finalize(res.results, x, y), res


def kernel(x, y):
    val, _ = run(x, y)
    return val


# revision 22
# speedup vs baseline: 1.7660x; 1.2271x over previous
"""Distance-correlation (DcorLoss) kernel for 8 trn2 NeuronCores.

Math: for x, y [n=8192, d=128]:
  a = pairwise_dist(x), b = pairwise_dist(y)   (n x n, symmetric, zero diag)
  A = double_center(a), B = double_center(b)
  dcor = -sqrt(sum(A*B)) / sqrt(sqrt(sum(A*A)) * sqrt(sum(B*B)))

Identities (never materialize A/B):
  sum(HaH o HbH) = sum(at o bt) - 2/n * dot(rs_at, rs_bt) + sum(at)*sum(bt)/n^2
with at = a - mu. sum(a-mu)^2 via the closed form for sum a^2 = sum sq.
Only sum (a-mu)*b and the row sums of a/b need streaming the matrices.

Symmetric block coverage: core c owns row block c (1024 rows). Each unordered
block pair {r, j} is computed once: core c runs 5 column-window "slots"
s=0..4 over windows (c+s) mod 8. Slot 0 = diagonal block, slots 1-3 pairs
counted twice on host, slot 4 pair computed by both ends (counted once each).
Row sums for the mirrored (uncomputed) windows of block c come from COLUMN
sums of slots 1-3 tiles of cores (c+5..c+7) mod 8, computed on-device with
ones^T matmuls accumulated in PSUM across the 8 row chunks of a slot.

Per (128-row x 1024-col) tile pair, the device computes:
  PE:   psum = -2*x_blk^T x  (fp32r, full speed at 512 moving cols)
        + column norms via fp8e4 DoubleRow matmuls (4-term hi/lo split rows)
        + mu^2*I on the slot-0 diagonal sub-block (fp8 DoubleRow)
  ACT:  t = sqrt(psum + n_i)  [per-partition fp32 row-norm bias]
  DVE:  row-sum reduces of t_a, t_b -> res columns
  POOL: (t_a - mu) * t_b with accum -> res columns (gpsimd engine)
  PE:   ones^T t_a / ones^T t_b column sums (slots 1-3, fp32r) -> PSUM
Cross-core combining is fp64 on host (partials are tiny).
"""

import os

import numpy as np
import ml_dtypes

import concourse.bass as bass
import concourse.tile as tile
from concourse import bacc, mybir
from concourse.bass_utils import run_bass_kernel_spmd

P = 128            # partitions / d
N = 8192           # points
NCORES = 8
BLK = N // NCORES  # 1024 rows per core
CI_N = BLK // P    # 8 row chunks per core
W = 1024           # column window
NSLOT = 5          # symmetric coverage slots
MU = 16.0
F8 = ml_dtypes.float8_e4m3
BF = ml_dtypes.bfloat16

_programs = {}


def _build():
    dt = mybir.dt
    f32 = dt.float32
    f32r = dt.float32r
    f8 = dt.float8e4
    bf = dt.bfloat16
    A = mybir.AluOpType
    AF = mybir.ActivationFunctionType
    DR = mybir.MatmulPerfMode.DoubleRow

    nc = bacc.Bacc("TRN2", target_bir_lowering=False, debug=False,
                   num_devices=NCORES)

    dxT = nc.dram_tensor("xT", [P, NSLOT * W], bf, kind="ExternalInput").ap()
    dyT = nc.dram_tensor("yT", [P, NSLOT * W], bf, kind="ExternalInput").ap()
    dxb = nc.dram_tensor("xb2", [P, BLK], bf, kind="ExternalInput").ap()
    dyb = nc.dram_tensor("yb2", [P, BLK], bf, kind="ExternalInput").ap()
    drn = nc.dram_tensor("rn", [P, 2 * CI_N], f32, kind="ExternalInput").ap()
    dcnx = nc.dram_tensor("cnx", [2, NSLOT * W], bf, kind="ExternalInput").ap()
    dcny = nc.dram_tensor("cny", [2, NSLOT * W], bf, kind="ExternalInput").ap()
    dey = nc.dram_tensor("eyew", [P, 2 * 384], f8, kind="ExternalInput").ap()
    dres = nc.dram_tensor("res", [P, 128], f32, kind="ExternalOutput").ap()
    dcols = nc.dram_tensor("cols", [1, 12 * 512], f32, kind="ExternalOutput").ap()

    with tile.TileContext(nc) as tc:
        with tc.tile_pool(name="const", bufs=1) as cp, \
             tc.tile_pool(name="psum", bufs=1, space="PSUM") as pp, \
             tc.tile_pool(name="ab", bufs=3) as abp, \
             tc.tile_pool(name="trd", bufs=2) as trd:

            # ── persistent operands ────────────────────────────────────
            xTt = cp.tile([P, NSLOT * W], bf, tag="xTt")
            yTt = cp.tile([P, NSLOT * W], bf, tag="yTt")
            xb2 = cp.tile([P, BLK], bf, tag="xb2")
            yb2 = cp.tile([P, BLK], bf, tag="yb2")
            rnt = cp.tile([P, 2 * CI_N], f32, tag="rnt")
            cntx = cp.tile([2, NSLOT * W], bf, tag="cntx")
            cnty = cp.tile([2, NSLOT * W], bf, tag="cnty")
            eyew = cp.tile([P, 2, 384], f8, tag="eyew")
            onesf = cp.tile([P, 1], f32, tag="onesf")
            nc.vector.memset(onesf[:], 1.0)
            ones1 = cp.tile([P, 1], f32r, tag="ones1")   # colsum lhsT (f32r)
            nc.vector.tensor_copy(ones1[:], onesf[:])
            res = cp.tile([P, 128], f32, tag="res")
            nc.vector.memset(res[:], 0.0)
            colstage = cp.tile([1, 12 * 512], f32, tag="colstage")
            nc.vector.memset(colstage[:], 0.0)

            # ACT sqrt-table preload (avoid a mid-loop ACT_TABLE_LOAD)
            sone = cp.tile([P, 1], f32, tag="sone")
            nc.vector.memset(sone[:], 1.0)
            sdum = cp.tile([P, 1], f32, tag="sdum")
            nc.scalar.activation(sdum[:], sone[:], AF.Sqrt)

            # PE warm-up: dense back-to-back matmuls on constant data so the
            # HAM clock reaches full speed before real work starts
            wur = cp.tile([2, 512], bf, tag="wur")
            nc.vector.memset(wur[:], 0.0)
            ones2b = cp.tile([2, P], bf, tag="ones2b")
            nc.vector.memset(ones2b[:], 1.0)
            wt = pp.tile([P, W], f32, tag="a", bufs=1)
            for _ in range(24):
                nc.tensor.matmul(wt[:, 0:512], ones2b[:], wur[:],
                                 start=True, stop=True)

            # ── input DMAs (small/critical first; windows stream in) ──
            nc.sync.dma_start(rnt[:], drn[:])
            nc.sync.dma_start(cntx[:], dcnx[:])
            nc.sync.dma_start(cnty[:], dcny[:])
            nc.sync.dma_start(eyew[:], dey[:])
            nc.sync.dma_start(xb2[:], dxb[:])
            nc.sync.dma_start(yb2[:], dyb[:])
            for s in range(NSLOT):
                sl = bass.ts(s, W)
                nc.sync.dma_start(xTt[:, sl], dxT[:, sl])
                nc.sync.dma_start(yTt[:, sl], dyT[:, sl])

            # ── main loop ─────────────────────────────────────────────
            for s in range(NSLOT):
                cst = None
                if 1 <= s <= 3:
                    cst = [pp.tile([1, 512], f32, tag=f"cs{k}", bufs=1,
                                   name=f"cs{k}")
                           for k in range(4)]
                for ci in range(CI_N):
                    col = s * CI_N + ci
                    psA = pp.tile([P, W], f32, tag="a", bufs=1)
                    psB = pp.tile([P, W], f32, tag="b", bufs=1)
                    for ps_, blk2, full, cnt in ((psA, xb2, xTt, cntx),
                                                 (psB, yb2, yTt, cnty)):
                        for h in range(2):
                            nc.tensor.matmul(
                                ps_[:, bass.ds(h * 512, 512)],
                                blk2[:, bass.ts(ci, P)],
                                full[:, bass.ds(s * W + h * 512, 512)],
                                start=True, stop=False)
                        if s == 0:
                            # += mu^2*I on this chunk's diagonal sub-block
                            qd = ci // 2
                            off = 128 * ((ci + 1) % 2)
                            nc.tensor.matmul(
                                ps_[:, bass.ds(qd * 256, 256)],
                                eyew[:, :, 128:256],
                                eyew[:, :, bass.ds(off, 256)],
                                start=False, stop=False, perf_mode=DR)
                        for h in range(2):
                            nc.tensor.matmul(
                                ps_[:, bass.ds(h * 512, 512)],
                                ones2b[:],
                                cnt[:, bass.ds(s * W + h * 512, 512)],
                                start=False, stop=True)

                    aT = abp.tile([P, W], f32r, tag="a")
                    bT = abp.tile([P, W], f32r, tag="b")
                    nc.scalar.activation(aT[:], psA[:], AF.Sqrt,
                                         bias=rnt[:, ci:ci + 1],
                                         accum_out=res[:, col:col + 1])
                    nc.scalar.activation(bT[:], psB[:], AF.Sqrt,
                                         bias=rnt[:, CI_N + ci:CI_N + ci + 1])
                    nc.vector.tensor_reduce(res[:, 40 + col:41 + col], bT[:],
                                            axis=mybir.AxisListType.X, op=A.add)
                    t0 = trd.tile([P, W], f32, tag="t")
                    nc.vector.scalar_tensor_tensor(
                        t0[:], aT[:], MU, bT[:], op0=A.subtract, op1=A.mult,
                        accum_out=res[:, 80 + col:81 + col])
                    if cst is not None:
                        for r, (ssrc, h) in enumerate(
                                ((aT, 0), (aT, 1), (bT, 0), (bT, 1))):
                            nc.tensor.matmul(
                                cst[r][:],
                                ones1[:],
                                ssrc[:, bass.ts(h, 512)],
                                start=(ci == 0), stop=(ci == CI_N - 1),
                                skip_group_check=True,
                                tile_position=(0, 0))
                if cst is not None:
                    for r in range(4):
                        off = (s - 1) * 2048 + r * 512
                        nc.vector.tensor_copy(
                            colstage[0:1, bass.ds(off, 512)],
                            cst[r][:])

            nc.sync.dma_start(dres[:], res[:])
            nc.sync.dma_start(dcols[:], colstage[:])

    nc.compile()
    return nc


def _get_program(mm_mode="f32r"):
    if mm_mode not in _programs:
        _programs[mm_mode] = _build()
    return _programs[mm_mode]


def _bf16_terms(v, k=2):
    """Successive bf16 split: v ~= sum of k bf16-representable terms."""
    r = np.asarray(v, np.float64).copy()
    terms = []
    for _ in range(k):
        t = r.astype(BF).astype(np.float64)
        terms.append(t)
        r -= t
    return terms


def _host_quant(x):
    """Per-matrix host-side quantities (fp64): norms and fp8 colnorm terms.

    The device consumes bf16(x); all norms come from those exact values."""
    x64 = np.asarray(x, np.float32).astype(BF).astype(np.float64)
    n_exact = (x64 * x64).sum(1)                       # [N]
    rn = n_exact.astype(np.float32).astype(np.float64)  # shipped fp32 bias
    terms = _bf16_terms(n_exact, 2)
    cn = terms[0] + terms[1]
    return n_exact, rn, terms, cn


def make_in_maps(x, y):
    x = np.ascontiguousarray(np.asarray(x, np.float32))
    y = np.ascontiguousarray(np.asarray(y, np.float32))
    _, rnx, tx, _ = _host_quant(x)
    _, rny, ty, _ = _host_quant(y)
    xT = x.astype(BF).T  # [128, 8192] bf16
    yT = y.astype(BF).T

    # eyew[p, 0, k] = 16*delta(p == k-128); plane 1 zero
    eyew = np.zeros((P, 2, 384), np.float32)
    for p in range(P):
        eyew[p, 0, p + 128] = MU
    eyew8 = eyew.astype(F8).reshape(P, 2 * 384)

    in_maps = []
    for c in range(NCORES):
        wins = [(c + s) % NCORES for s in range(NSLOT)]
        colsel = np.concatenate([np.arange(w * W, (w + 1) * W) for w in wins])
        rn = np.empty((P, 2 * CI_N), np.float32)
        for ci in range(CI_N):
            base = c * BLK + ci * P
            rn[:, ci] = rnx[base:base + P]
            rn[:, CI_N + ci] = rny[base:base + P]

        def cn_pack(terms):
            # [2, NSLOT*W]: row 0 = hi, row 1 = lo (bf16)
            out = np.zeros((2, NSLOT * W), np.float32)
            out[0] = terms[0][colsel]
            out[1] = terms[1][colsel]
            return out.astype(BF)

        in_maps.append({
            "xT": np.ascontiguousarray(xT[:, colsel]),
            "yT": np.ascontiguousarray(yT[:, colsel]),
            "xb2": np.ascontiguousarray(
                (-2.0 * xT[:, c * BLK:(c + 1) * BLK].astype(np.float32))
                .astype(BF)),
            "yb2": np.ascontiguousarray(
                (-2.0 * yT[:, c * BLK:(c + 1) * BLK].astype(np.float32))
                .astype(BF)),
            "rn": rn,
            "cnx": cn_pack(tx),
            "cny": cn_pack(ty),
            "eyew": eyew8,
        })
    return in_maps


def finalize(outs, x, y):
    """outs: list of 8 dicts with 'res' [128,128] and 'cols' [4, 3072].

    res cols: rs_a 0:40 | rs_b 40:80 | pab 80:120, col = s*8+ci, value at
    partition p belongs to row c*1024+ci*128+p.
    cols rows: 0/1 = a-tile column sums (halves 0/1), 2/3 = same for b;
    slot s occupies cols (s-1)*512 : s*512.
    """
    n = float(N)
    nx, rnx, _, cnx = _host_quant(x)
    ny, rny, _, cny = _host_quant(y)
    x64 = np.asarray(x, np.float32).astype(BF).astype(np.float64)
    y64 = np.asarray(y, np.float32).astype(BF).astype(np.float64)

    res = [np.asarray(o["res"], np.float64) for o in outs]
    cols = [np.asarray(o["cols"], np.float64) for o in outs]

    rs_a = np.empty(N)
    rs_b = np.empty(N)
    pab = 0.0
    wslot = np.array([1.0, 2.0, 2.0, 2.0, 1.0])
    for c in range(NCORES):
        r = res[c]
        st0 = r[:, 0:40].reshape(P, NSLOT, CI_N)    # [p, s, ci]
        st1 = r[:, 40:80].reshape(P, NSLOT, CI_N)
        st2 = r[:, 80:120].reshape(P, NSLOT, CI_N)
        own_a = st0.sum(axis=1)                     # [p, ci]
        own_b = st1.sum(axis=1)
        # mirrored contributions: window (c+d)%8, d=5,6,7 -> core m slot 8-d
        mir_a = np.zeros(BLK)
        mir_b = np.zeros(BLK)
        for d in (5, 6, 7):
            m = (c + d) % NCORES
            sp = 8 - d
            base = (sp - 1) * 2048
            cv = cols[m][0]
            mir_a += np.concatenate([cv[base:base + 512],
                                     cv[base + 512:base + 1024]])
            mir_b += np.concatenate([cv[base + 1024:base + 1536],
                                     cv[base + 1536:base + 2048]])
        blk_a = own_a.T.ravel() + mir_a             # [1024], ci-major
        blk_b = own_b.T.ravel() + mir_b
        rs_a[c * BLK:(c + 1) * BLK] = blk_a
        rs_b[c * BLK:(c + 1) * BLK] = blk_b
        pab += (st2.sum(axis=(0, 2)) * wslot).sum()

    # closed-form sums of device sq over all ij (fp64, host-exact)
    sum_sq_a = n * rnx.sum() + n * cnx.sum() - 2.0 * (x64.sum(0) @ x64.sum(0))
    sum_sq_b = n * rny.sum() + n * cny.sum() - 2.0 * (y64.sum(0) @ y64.sum(0))
    diag_sq_a = (rnx + cnx - 2.0 * nx).sum()
    diag_sq_b = (rny + cny - 2.0 * ny).sum()

    sa = rs_a - MU          # true rowsums (device diag sqrt(256+eps) ~ 16)
    sb = rs_b - MU
    Sq_a_off = sum_sq_a - diag_sq_a
    Sq_b_off = sum_sq_b - diag_sq_b
    sat = sa - n * MU
    sbt = sb - n * MU
    Ua, Ub = sat.sum(), sbt.sum()
    # device pab = weighted sum of (a-mu)*b; forced diag contributes
    # (16-16)*16 = 0, matching the true (0-mu)*0 = 0.
    # Sab = sum over all ij of (a_true - mu)(b_true - mu)
    Sab = pab - MU * (sa.sum() - MU * n * n)
    Saa = Sq_a_off - 2.0 * MU * sa.sum() + MU * MU * n * n
    Sbb = Sq_b_off - 2.0 * MU * sb.sum() + MU * MU * n * n

    sumAB = Sab - 2.0 * np.dot(sat, sbt) / n + Ua * Ub / n ** 2
    sumAA = Saa - 2.0 * np.dot(sat, sat) / n + Ua * Ua / n ** 2
    sumBB = Sbb - 2.0 * np.dot(sbt, sbt) / n + Ub * Ub / n ** 2

    inv_n2 = 1.0 / (n * n)
    dcor = (-np.sqrt(sumAB * inv_n2)
            / np.sqrt(np.sqrt(sumAA * inv_n2) * np.sqrt(sumBB * inv_n2)))
    return np.asarray(dcor, dtype=np.float32)


def run(x, y, mm_mode=None, trace=False, tmpdir=None):
    nc = _get_program()
    in_maps = make_in_maps(x, y)
    res = run_bass_kernel_spmd(nc, in_maps, core_ids=list(range(NCORES)),
                               trace=trace, tmpdir=tmpdir)
    return finalize(res.results, x, y), res


def kernel(x, y):
    val, _ = run(x, y)
    return val


# revision 23
# speedup vs baseline: 1.8994x; 1.0755x over previous
"""Distance-correlation (DcorLoss) kernel for 8 trn2 NeuronCores.

Math: for x, y [n=8192, d=128]:
  a = pairwise_dist(x), b = pairwise_dist(y)   (n x n, symmetric, zero diag)
  A = double_center(a), B = double_center(b)
  dcor = -sqrt(sum(A*B)) / sqrt(sqrt(sum(A*A)) * sqrt(sum(B*B)))

Identities (never materialize A/B):
  sum(HaH o HbH) = sum(at o bt) - 2/n * dot(rs_at, rs_bt) + sum(at)*sum(bt)/n^2
with at = a - mu. sum(a-mu)^2 via the closed form for sum a^2 = sum sq.
Only sum (a-mu)*b and the row sums of a/b need streaming the matrices.

Symmetric block coverage: core c owns row block c (1024 rows). Each unordered
block pair {r, j} is computed once: core c runs 5 column-window "slots"
s=0..4 over windows (c+s) mod 8. Slot 0 = diagonal block, slots 1-3 pairs
counted twice on host, slot 4 pair computed by both ends (counted once each).
Row sums for the mirrored (uncomputed) windows of block c come from COLUMN
sums of slots 1-3 tiles of cores (c+5..c+7) mod 8, computed on-device with
ones^T matmuls accumulated in PSUM across the 8 row chunks of a slot.

Per (128-row x 1024-col) tile pair, the device computes:
  PE:   psum = -2*x_blk^T x  (fp32r, full speed at 512 moving cols)
        + column norms via fp8e4 DoubleRow matmuls (4-term hi/lo split rows)
        + mu^2*I on the slot-0 diagonal sub-block (fp8 DoubleRow)
  ACT:  t = sqrt(psum + n_i)  [per-partition fp32 row-norm bias]
  DVE:  row-sum reduces of t_a, t_b -> res columns
  POOL: (t_a - mu) * t_b with accum -> res columns (gpsimd engine)
  PE:   ones^T t_a / ones^T t_b column sums (slots 1-3, fp32r) -> PSUM
Cross-core combining is fp64 on host (partials are tiny).
"""

import os

import numpy as np
import ml_dtypes

import concourse.bass as bass
import concourse.tile as tile
from concourse import bacc, mybir
from concourse.bass_utils import run_bass_kernel_spmd

P = 128            # partitions / d
N = 8192           # points
NCORES = 8
BLK = N // NCORES  # 1024 rows per core
CI_N = BLK // P    # 8 row chunks per core
W = 1024           # column window
NSLOT = 5          # symmetric coverage slots
MU = 16.0
F8 = ml_dtypes.float8_e4m3
BF = ml_dtypes.bfloat16

_programs = {}


def _emit_cs(nc, cst, ones1, pend, ci_n):
    aT, bT, ci = pend
    for r, (ssrc, h) in enumerate(((aT, 0), (aT, 1), (bT, 0), (bT, 1))):
        nc.tensor.matmul(
            cst[r][:], ones1[:], ssrc[:, bass.ts(h, 512)],
            start=(ci == 0), stop=(ci == ci_n - 1),
            skip_group_check=True, tile_position=(0, 0))


def _build():
    dt = mybir.dt
    f32 = dt.float32
    f32r = dt.float32r
    f8 = dt.float8e4
    bf = dt.bfloat16
    A = mybir.AluOpType
    AF = mybir.ActivationFunctionType
    DR = mybir.MatmulPerfMode.DoubleRow

    nc = bacc.Bacc("TRN2", target_bir_lowering=False, debug=False,
                   num_devices=NCORES)

    dxT = nc.dram_tensor("xT", [P, NSLOT * W], bf, kind="ExternalInput").ap()
    dyT = nc.dram_tensor("yT", [P, NSLOT * W], bf, kind="ExternalInput").ap()
    dxb = nc.dram_tensor("xb2", [P, BLK], bf, kind="ExternalInput").ap()
    dyb = nc.dram_tensor("yb2", [P, BLK], bf, kind="ExternalInput").ap()
    drn = nc.dram_tensor("rn", [P, 2 * CI_N], f32, kind="ExternalInput").ap()
    dcnx = nc.dram_tensor("cnx", [2, NSLOT * W], bf, kind="ExternalInput").ap()
    dcny = nc.dram_tensor("cny", [2, NSLOT * W], bf, kind="ExternalInput").ap()
    dey = nc.dram_tensor("eyew", [P, 2 * 384], f8, kind="ExternalInput").ap()
    dres = nc.dram_tensor("res", [P, 128], f32, kind="ExternalOutput").ap()
    dcols = nc.dram_tensor("cols", [1, 12 * 512], f32, kind="ExternalOutput").ap()

    with tile.TileContext(nc) as tc:
        with tc.tile_pool(name="const", bufs=1) as cp, \
             tc.tile_pool(name="psum", bufs=1, space="PSUM") as pp, \
             tc.tile_pool(name="ab", bufs=3) as abp, \
             tc.tile_pool(name="trd", bufs=2) as trd:

            # ── persistent operands ────────────────────────────────────
            xTt = cp.tile([P, NSLOT * W], bf, tag="xTt")
            yTt = cp.tile([P, NSLOT * W], bf, tag="yTt")
            xb2 = cp.tile([P, BLK], bf, tag="xb2")
            yb2 = cp.tile([P, BLK], bf, tag="yb2")
            rnt = cp.tile([P, 2 * CI_N], f32, tag="rnt")
            cntx = cp.tile([2, NSLOT * W], bf, tag="cntx")
            cnty = cp.tile([2, NSLOT * W], bf, tag="cnty")
            eyew = cp.tile([P, 2, 384], f8, tag="eyew")
            onesf = cp.tile([P, 1], f32, tag="onesf")
            nc.vector.memset(onesf[:], 1.0)
            ones1 = cp.tile([P, 1], f32r, tag="ones1")   # colsum lhsT (f32r)
            nc.vector.tensor_copy(ones1[:], onesf[:])
            res = cp.tile([P, 128], f32, tag="res")
            nc.vector.memset(res[:], 0.0)
            colstage = cp.tile([1, 12 * 512], f32, tag="colstage")

            # ACT sqrt-table preload (avoid a mid-loop ACT_TABLE_LOAD)
            sone = cp.tile([P, 1], f32, tag="sone")
            nc.vector.memset(sone[:], 1.0)
            sdum = cp.tile([P, 1], f32, tag="sdum")
            nc.scalar.activation(sdum[:], sone[:], AF.Sqrt)

            # PE warm-up: dense back-to-back matmuls on constant data so the
            # HAM clock reaches full speed before real work starts
            wur = cp.tile([2, 512], bf, tag="wur")
            nc.vector.memset(wur[:], 0.0)
            ones2b = cp.tile([2, P], bf, tag="ones2b")
            nc.vector.memset(ones2b[:], 1.0)
            wt = pp.tile([P, W], f32, tag="a", bufs=1)
            for _ in range(12):
                nc.tensor.matmul(wt[:, 0:512], ones2b[:], wur[:],
                                 start=True, stop=True)

            # ── input DMAs (small/critical first; windows stream in) ──
            nc.sync.dma_start(rnt[:], drn[:])
            nc.sync.dma_start(cntx[:], dcnx[:])
            nc.sync.dma_start(cnty[:], dcny[:])
            nc.sync.dma_start(eyew[:], dey[:])
            nc.sync.dma_start(xb2[:], dxb[:])
            nc.sync.dma_start(yb2[:], dyb[:])
            for s in range(NSLOT):
                sl = bass.ts(s, W)
                nc.sync.dma_start(xTt[:, sl], dxT[:, sl])
                nc.sync.dma_start(yTt[:, sl], dyT[:, sl])

            # ── main loop ─────────────────────────────────────────────
            for s in range(NSLOT):
                cst = None
                if 1 <= s <= 3:
                    cst = [pp.tile([1, 512], f32, tag=f"cs{k}", bufs=1,
                                   name=f"cs{k}")
                           for k in range(4)]
                pend = None
                for ci in range(CI_N):
                    col = s * CI_N + ci
                    if pend is not None:
                        _emit_cs(nc, cst, ones1, pend, CI_N)
                        pend = None
                    psA = pp.tile([P, W], f32, tag="a", bufs=1)
                    psB = pp.tile([P, W], f32, tag="b", bufs=1)
                    for ps_, blk2, full, cnt in ((psA, xb2, xTt, cntx),
                                                 (psB, yb2, yTt, cnty)):
                        for h in range(2):
                            nc.tensor.matmul(
                                ps_[:, bass.ds(h * 512, 512)],
                                blk2[:, bass.ts(ci, P)],
                                full[:, bass.ds(s * W + h * 512, 512)],
                                start=True, stop=False)
                        if s == 0:
                            # += mu^2*I on this chunk's diagonal sub-block
                            qd = ci // 2
                            off = 128 * ((ci + 1) % 2)
                            nc.tensor.matmul(
                                ps_[:, bass.ds(qd * 256, 256)],
                                eyew[:, :, 128:256],
                                eyew[:, :, bass.ds(off, 256)],
                                start=False, stop=False, perf_mode=DR)
                        for h in range(2):
                            nc.tensor.matmul(
                                ps_[:, bass.ds(h * 512, 512)],
                                ones2b[:],
                                cnt[:, bass.ds(s * W + h * 512, 512)],
                                start=False, stop=True)

                    aT = abp.tile([P, W], f32r, tag="a")
                    bT = abp.tile([P, W], f32r, tag="b")
                    nc.scalar.activation(aT[:], psA[:], AF.Sqrt,
                                         bias=rnt[:, ci:ci + 1],
                                         accum_out=res[:, col:col + 1])
                    nc.scalar.activation(bT[:], psB[:], AF.Sqrt,
                                         bias=rnt[:, CI_N + ci:CI_N + ci + 1])
                    nc.vector.tensor_reduce(res[:, 40 + col:41 + col], bT[:],
                                            axis=mybir.AxisListType.X, op=A.add)
                    t0 = trd.tile([P, W], f32, tag="t")
                    nc.vector.scalar_tensor_tensor(
                        t0[:], aT[:], MU, bT[:], op0=A.subtract, op1=A.mult,
                        accum_out=res[:, 80 + col:81 + col])
                    if cst is not None:
                        pend = (aT, bT, ci)
                if cst is not None:
                    _emit_cs(nc, cst, ones1, pend, CI_N)
                    for r in range(4):
                        off = (s - 1) * 2048 + r * 512
                        nc.vector.tensor_copy(
                            colstage[0:1, bass.ds(off, 512)],
                            cst[r][:])

            nc.sync.dma_start(dres[:], res[:])
            nc.sync.dma_start(dcols[:], colstage[:])

    nc.compile()
    return nc


def _get_program(mm_mode="f32r"):
    if mm_mode not in _programs:
        _programs[mm_mode] = _build()
    return _programs[mm_mode]


def _bf16_terms(v, k=2):
    """Successive bf16 split: v ~= sum of k bf16-representable terms."""
    r = np.asarray(v, np.float64).copy()
    terms = []
    for _ in range(k):
        t = r.astype(BF).astype(np.float64)
        terms.append(t)
        r -= t
    return terms


def _host_quant(x):
    """Per-matrix host-side quantities (fp64): norms and fp8 colnorm terms.

    The device consumes bf16(x); all norms come from those exact values."""
    x64 = np.asarray(x, np.float32).astype(BF).astype(np.float64)
    n_exact = (x64 * x64).sum(1)                       # [N]
    rn = n_exact.astype(np.float32).astype(np.float64)  # shipped fp32 bias
    terms = _bf16_terms(n_exact, 2)
    cn = terms[0] + terms[1]
    return n_exact, rn, terms, cn


def make_in_maps(x, y):
    x = np.ascontiguousarray(np.asarray(x, np.float32))
    y = np.ascontiguousarray(np.asarray(y, np.float32))
    _, rnx, tx, _ = _host_quant(x)
    _, rny, ty, _ = _host_quant(y)
    xT = x.astype(BF).T  # [128, 8192] bf16
    yT = y.astype(BF).T

    # eyew[p, 0, k] = 16*delta(p == k-128); plane 1 zero
    eyew = np.zeros((P, 2, 384), np.float32)
    for p in range(P):
        eyew[p, 0, p + 128] = MU
    eyew8 = eyew.astype(F8).reshape(P, 2 * 384)

    in_maps = []
    for c in range(NCORES):
        wins = [(c + s) % NCORES for s in range(NSLOT)]
        colsel = np.concatenate([np.arange(w * W, (w + 1) * W) for w in wins])
        rn = np.empty((P, 2 * CI_N), np.float32)
        for ci in range(CI_N):
            base = c * BLK + ci * P
            rn[:, ci] = rnx[base:base + P]
            rn[:, CI_N + ci] = rny[base:base + P]

        def cn_pack(terms):
            # [2, NSLOT*W]: row 0 = hi, row 1 = lo (bf16)
            out = np.zeros((2, NSLOT * W), np.float32)
            out[0] = terms[0][colsel]
            out[1] = terms[1][colsel]
            return out.astype(BF)

        in_maps.append({
            "xT": np.ascontiguousarray(xT[:, colsel]),
            "yT": np.ascontiguousarray(yT[:, colsel]),
            "xb2": np.ascontiguousarray(
                (-2.0 * xT[:, c * BLK:(c + 1) * BLK].astype(np.float32))
                .astype(BF)),
            "yb2": np.ascontiguousarray(
                (-2.0 * yT[:, c * BLK:(c + 1) * BLK].astype(np.float32))
                .astype(BF)),
            "rn": rn,
            "cnx": cn_pack(tx),
            "cny": cn_pack(ty),
            "eyew": eyew8,
        })
    return in_maps


def finalize(outs, x, y):
    """outs: list of 8 dicts with 'res' [128,128] and 'cols' [4, 3072].

    res cols: rs_a 0:40 | rs_b 40:80 | pab 80:120, col = s*8+ci, value at
    partition p belongs to row c*1024+ci*128+p.
    cols rows: 0/1 = a-tile column sums (halves 0/1), 2/3 = same for b;
    slot s occupies cols (s-1)*512 : s*512.
    """
    n = float(N)
    nx, rnx, _, cnx = _host_quant(x)
    ny, rny, _, cny = _host_quant(y)
    x64 = np.asarray(x, np.float32).astype(BF).astype(np.float64)
    y64 = np.asarray(y, np.float32).astype(BF).astype(np.float64)

    res = [np.asarray(o["res"], np.float64) for o in outs]
    cols = [np.asarray(o["cols"], np.float64) for o in outs]

    rs_a = np.empty(N)
    rs_b = np.empty(N)
    pab = 0.0
    wslot = np.array([1.0, 2.0, 2.0, 2.0, 1.0])
    for c in range(NCORES):
        r = res[c]
        st0 = r[:, 0:40].reshape(P, NSLOT, CI_N)    # [p, s, ci]
        st1 = r[:, 40:80].reshape(P, NSLOT, CI_N)
        st2 = r[:, 80:120].reshape(P, NSLOT, CI_N)
        own_a = st0.sum(axis=1)                     # [p, ci]
        own_b = st1.sum(axis=1)
        # mirrored contributions: window (c+d)%8, d=5,6,7 -> core m slot 8-d
        mir_a = np.zeros(BLK)
        mir_b = np.zeros(BLK)
        for d in (5, 6, 7):
            m = (c + d) % NCORES
            sp = 8 - d
            base = (sp - 1) * 2048
            cv = cols[m][0]
            mir_a += np.concatenate([cv[base:base + 512],
                                     cv[base + 512:base + 1024]])
            mir_b += np.concatenate([cv[base + 1024:base + 1536],
                                     cv[base + 1536:base + 2048]])
        blk_a = own_a.T.ravel() + mir_a             # [1024], ci-major
        blk_b = own_b.T.ravel() + mir_b
        rs_a[c * BLK:(c + 1) * BLK] = blk_a
        rs_b[c * BLK:(c + 1) * BLK] = blk_b
        pab += (st2.sum(axis=(0, 2)) * wslot).sum()

    # closed-form sums of device sq over all ij (fp64, host-exact)
    sum_sq_a = n * rnx.sum() + n * cnx.sum() - 2.0 * (x64.sum(0) @ x64.sum(0))
    sum_sq_b = n * rny.sum() + n * cny.sum() - 2.0 * (y64.sum(0) @ y64.sum(0))
    diag_sq_a = (rnx + cnx - 2.0 * nx).sum()
    diag_sq_b = (rny + cny - 2.0 * ny).sum()

    sa = rs_a - MU          # true rowsums (device diag sqrt(256+eps) ~ 16)
    sb = rs_b - MU
    Sq_a_off = sum_sq_a - diag_sq_a
    Sq_b_off = sum_sq_b - diag_sq_b
    sat = sa - n * MU
    sbt = sb - n * MU
    Ua, Ub = sat.sum(), sbt.sum()
    # device pab = weighted sum of (a-mu)*b; forced diag contributes
    # (16-16)*16 = 0, matching the true (0-mu)*0 = 0.
    # Sab = sum over all ij of (a_true - mu)(b_true - mu)
    Sab = pab - MU * (sa.sum() - MU * n * n)
    Saa = Sq_a_off - 2.0 * MU * sa.sum() + MU * MU * n * n
    Sbb = Sq_b_off - 2.0 * MU * sb.sum() + MU * MU * n * n

    sumAB = Sab - 2.0 * np.dot(sat, sbt) / n + Ua * Ub / n ** 2
    sumAA = Saa - 2.0 * np.dot(sat, sat) / n + Ua * Ua / n ** 2
    sumBB = Sbb - 2.0 * np.dot(sbt, sbt) / n + Ub * Ub / n ** 2

    inv_n2 = 1.0 / (n * n)
    dcor = (-np.sqrt(sumAB * inv_n2)
            / np.sqrt(np.sqrt(sumAA * inv_n2) * np.sqrt(sumBB * inv_n2)))
    return np.asarray(dcor, dtype=np.float32)


def run(x, y, mm_mode=None, trace=False, tmpdir=None):
    nc = _get_program()
    in_maps = make_in_maps(x, y)
    res = run_bass_kernel_spmd(nc, in_maps, core_ids=list(range(NCORES)),
                               trace=trace, tmpdir=tmpdir)
    return finalize(res.results, x, y), res


def kernel(x, y):
    val, _ = run(x, y)
    return val
